# revision 60
# baseline (speedup 1.0000x reference)
"""Trainium2 Bass kernel for BroadcastResidualBlock.

Reference computation (per image, NHWC, H=W=19, C=256, HW=361):
    h1 = relu(bn1(x @ conv1_w + conv1_b))          # 1x1 conv = channel mix
    h2 = relu(dense(h1 over flattened board))       # spatial mix, per channel
    h3 = relu(bn2(h2 @ conv2_w + conv2_b))          # 1x1 conv
    out = x + h3

Strategy: pure data parallel over batch N=256 -> 32 images per core on 8
cores.  BN (inference) folds into the conv weights/biases on the host.

Matmul precision/speed: stages 1 and 2 run as 3-term hi/lo fp8(e4m3)
DoubleRow matmuls.  Each operand A is split A = Ah + Al (both fp8, computed
by round-to-nearest so |A - Ah - Al| <~ 2^-8 |A|), and A@B is evaluated as
Ah@Bh + Al@Bh + Ah@Bl (the dropped Al@Bl term is ~2^-8 relative).  A
DoubleRow instruction computes lhsT[:,0].T@rhs[:,0] + lhsT[:,1].T@rhs[:,1]
at 0.5 PE-cycles per moving column, so one instruction folds two 128-deep
k-tiles: a k=256 3-term product costs 3 half-rate instructions vs 2
full-rate bf16 ones (1.33x), and s2's 9 k-tile-term products pack into 5
instructions vs 6 bf16 (1.2x).  Stage 3 stays bf16 (its hi/lo split would
need a 5th PSUM-reading vector op per image, which doesn't fit the
ACT+DVE budget).  Residual adds bf16 x; output is stored bf16.

Per-image device work (cost-model):
    PE:  s1 9 DR insts (480ns) + s2 10 DR (752ns) + s3 4 bf16 (602ns)
    ACT: h1 relu->fp8 hi (825ns), h2 relu->bf16 (787ns)
    DVE: h1 lo extract (924ns), s3 relu+residual (877ns)
DMA: x fp8 hi/lo [N,P,4,HW] + x bf16 [N,P,2,HW] in, y bf16 out.

Schedule: 4-deep software pipeline over images (s1(t) | s2(t-2) | s3(t-3))
so each matmul->ACT->DVE epilogue chain (~2.7us) has a full step of slack
before its consumer.  PSUM: s1/s3 share a 3-slot ring of 2-bank tiles;
s2's two c-chunks get their own 2-slot ring of 1-bank tiles (8 banks
total) so s2 matmuls never wait on a slot freed by the saturated DVE.
x loads ride the sync queue (first few split over sync+scalar), steady
stores the gpsimd (SWDGE) queue, tail stores sync/scalar per d-chunk.
Cost-model timeline 69.5us vs 80.8us for the all-bf16 predecessor
(measured rel err on hardware: 8.0e-3, budget 2e-2); PE busy 58.6us
(84%), DVE 57.7, ACT 52.9, DMA 51.3.
"""

import numpy as np
import ml_dtypes

import concourse.bass as bass
import concourse.mybir as mybir
import concourse.tile as tile
from concourse import bacc
from concourse.bass_utils import run_bass_kernel_spmd

N_CORES = 8
NIMG = 32            # images per core
C = 256
HW = 361             # 19*19
XW = 384             # q zero-padded: even slot stride (dual-fp8 Ldweights
                     # restriction) AND room for a full m=128 third s1 chunk so
                     # every psum row is written (stale-psum reads are
                     # nondeterministic garbage on a fresh device)
P = 128
EPS = 1e-3

F32 = mybir.dt.float32
BF16 = mybir.dt.bfloat16
F8 = mybir.dt.float8e4
AF = mybir.ActivationFunctionType
ALU = mybir.AluOpType
DR = mybir.MatmulPerfMode.DoubleRow

# DMA batches: singles at the edges (short critical path at startup/teardown),
# pairs in steady state
BATCHES = ([[0], [1], [2], [3]] + [[i, i + 1] for i in range(4, 30, 2)]
           + [[30], [31]])
BMAX = 2

_prog_cache = {}

POOL_CFG = dict(xf=7, xb=6, h1=8, h2=5, yo=4)
# startup emission order: (token, queue) — x8 batch indices and weight-chunk
# tokens.  Interleaving sync/scalar doubles the DMA issue rate (each queue
# issues one DMA per ~1.2us of SEQ+HWDGE config); the scalar queue must go
# quiet before ~4.5us so ACT epilogue dispatch isn't blocked.
LOAD_PLAN = ((0, "sy"), (1, "sy"), (2, "sc"), ("dwh", "sy"), (3, "sc"),
             ("dwl", "sy"), (4, "sy"), ("w2", "sy"))
# per-step emission order of matmul groups: (stage, group).  The pipeline is
# 4-deep (s1(t) | s2(t-2) | s3(t-3)) so each epilogue chain (matmul -> ACT ->
# DVE, ~2.7us) has a full step of slack before its consumer.  s1 first: its
# psum fills early, freeing ACT to start; s3 last: its psum tile reuses the
# slot h1-lo releases mid-step.
STEP_ORDER = [(1,0),(1,1),(1,2),(2,0),(2,1),(3,0),(3,1)]
# fill steps: s2/s3 groups first — their inputs are ready while s1 often
# still waits on an x8 batch DMA, and the PE queue is in-order (head-of-line)
EARLY_ORDER = [(2,0),(2,1),(3,0),(3,1),(1,0),(1,1),(1,2)]
EARLY_CUT = 0

# s2 DoubleRow slot pairing: 3-term hi/lo over 3 k-tiles (third zero-padded
# to 128 rows on the dw side, which also nullifies the h1 pad rows).
# h1 slots: [hh0 hh1 hh2 lh0 lh1 lh2]; dw slots:
# [dwh0 dwh1 dwh2 dwh0 dwh1 dwh2 dwl0 dwl1 dwl2 zero]
# hi-only pairs first: they depend only on the ACT half of the h1 epilogue
S2_PAIRS = [(0, 0), (2, 2), (4, 4), (0, 6), (2, 8)]

# LO_TILES=3: full lo-correction (rel err ~8e-3).  LO_TILES=2: drop the lo
# term for k-tile 2 (the 105-row remainder, 29% of the contraction): the DVE
# h1-lo pass shrinks 768->512 cols, giving DVE slack under the PE period
# (rel err ~1.4e-2, still 30% under the 2e-2 budget).
LO_TILES = 3
DRAIN_S2_SPLIT = False
DRAIN_S3_ILV = False


def build_program(has_b1: bool, has_b2: bool, has_b3: bool, reps: int = 1):
    nc = bacc.Bacc("TRN2", target_bir_lowering=False, debug=False)

    x8 = nc.dram_tensor("x8", [NIMG, P, 4, XW], F8, kind="ExternalInput").ap()
    xb = nc.dram_tensor("xb", [NIMG, P, 2, HW], BF16, kind="ExternalInput").ap()
    w1b = nc.dram_tensor("w1b", [P, 4, C], F8, kind="ExternalInput").ap()
    dwb = nc.dram_tensor("dwb", [P, 10, XW], F8, kind="ExternalInput").ap()
    w2b = nc.dram_tensor("w2b", [P, 2, C], BF16, kind="ExternalInput").ap()
    b1 = b2 = b3 = None
    if has_b1:
        b1 = nc.dram_tensor("b1", [P, 3 * C], F32, kind="ExternalInput").ap()
    if has_b2:
        b2 = nc.dram_tensor("b2", [P, 2, HW], F32, kind="ExternalInput").ap()
    if has_b3:
        b3 = nc.dram_tensor("b3", [2, P], F32, kind="ExternalInput").ap()
    yc = nc.dram_tensor("yc", [NIMG, P, 2, HW], BF16, kind="ExternalOutput").ap()

    batch_of = {}
    for bi, imgs in enumerate(BATCHES):
        for k, i in enumerate(imgs):
            batch_of[i] = (bi, k)

    with tile.TileContext(nc) as tc:
        with (
            tc.tile_pool(name="const", bufs=1) as cpool,
            tc.tile_pool(name="xf", bufs=POOL_CFG["xf"]) as xf_pool,
            tc.tile_pool(name="xbp", bufs=POOL_CFG["xb"]) as xb_pool,
            tc.tile_pool(name="h1", bufs=POOL_CFG["h1"]) as h1_pool,
            tc.tile_pool(name="h2", bufs=POOL_CFG["h2"]) as h2_pool,
            tc.tile_pool(name="yo", bufs=POOL_CFG["yo"]) as yo_pool,
            tc.tile_pool(name="ps", bufs=3, space="PSUM") as ps_pool,
            tc.tile_pool(name="ps2", bufs=2, space="PSUM") as ps2_pool,
        ):
            # weights: w1 ships first so stage-1 matmuls unblock as early as
            # possible; dw/w2 DMAs are emitted inside body() AFTER the first
            # x8 loads so they don't delay those on the scalar queue
            # w1 rides the SWDGE (gpsimd) queue: Pool.SEQ is idle at t=0 and
            # its issue chain is ~600ns shorter than HWDGE, so the very first
            # stage-1 matmul unblocks earlier
            w1sb = cpool.tile([P, 4, C], F8)
            nc.gpsimd.dma_start(w1sb[:], w1b)
            dwsb = cpool.tile([P, 10, XW], F8)
            w2sb = cpool.tile([P, 2, C], BF16)

            b1sb = b2sb = b3sb = None
            if has_b1:
                b1sb = cpool.tile([P, 3 * C], F32)
            if has_b2:
                b2sb = cpool.tile([P, 2, HW], F32)
            if has_b3:
                b3sb = cpool.tile([P, 2], F32)

            def emit_s1_group(i, xf, k, rc, h1, pss):
                ps = pss["s1"]
                # all three chunks run m=128: rows 105..127 of chunk 2 read
                # zero-padded x columns, so the psum (and h1's fp8 hi/lo pad
                # rows) get written-zero instead of stale garbage
                out = ps[:, rc * C : rc * C + C]
                csl = slice(rc * 128, rc * 128 + 128)
                # the two w1-hi terms first: the first matmuls of the
                # program only need the first half of the w1 blob
                nc.tensor.matmul(out, xf[:, k, 0:2, csl], w1sb[:, 0:2, :],
                                 start=True, stop=False, perf_mode=DR)
                nc.tensor.matmul(out, xf[:, k, 2:4, csl], w1sb[:, 0:2, :],
                                 start=False, stop=False, perf_mode=DR)
                nc.tensor.matmul(out, xf[:, k, 0:2, csl], w1sb[:, 2:4, :],
                                 start=False, stop=True, perf_mode=DR)
                if rc < 2:
                    return
                # epilogue: hi on ACT, lo on DVE
                if b1sb is not None:
                    nc.vector.scalar_tensor_tensor(
                        ps[:, : 3 * C], ps[:, : 3 * C], 0.0, b1sb[:],
                        ALU.bypass, ALU.add)
                hflat = h1[:, 0:3, :].rearrange("p a b -> p (a b)")
                nc.scalar.activation(hflat, ps[:, : 3 * C], AF.Relu)
                nl = LO_TILES
                lflat = h1[:, 3 : 3 + nl, :].rearrange("p a b -> p (a b)")
                hpart = h1[:, 0:nl, :].rearrange("p a b -> p (a b)")
                nc.vector.scalar_tensor_tensor(
                    lflat, ps[:, : nl * C], 0.0, hpart, ALU.max, ALU.subtract)

            def emit_s2_group(i, h1, cc, h2, pss):
                # each c-chunk gets its own 1-bank psum from the s2 ring, and
                # its own relu: decouples s2 matmuls from the s1/s3 psum ring
                # (whose slots are freed by the heavily-loaded DVE) and lets
                # h2's first half reach s3 a half-step early
                ps = pss["s2a" if cc == 0 else "s2b"]
                out = ps[:, :HW]
                csl = slice(cc * 128, (cc + 1) * 128)
                last = len(S2_PAIRS) - 1
                for idx, (hs, ds) in enumerate(S2_PAIRS):
                    nc.tensor.matmul(out, h1[:, hs : hs + 2, csl],
                                     dwsb[:, ds : ds + 2, :HW],
                                     start=(idx == 0), stop=(idx == last),
                                     perf_mode=DR)
                if b2sb is not None:
                    nc.vector.scalar_tensor_tensor(
                        out, out, 0.0, b2sb[:, cc, :], ALU.bypass, ALU.add)
                nc.scalar.activation(h2[:, cc, :], out, AF.Relu)

            def emit_s3_drain(i, k, yo, h2, xbt):
                # tail images: separate 1-bank psums per d-chunk, k-chunks
                # interleaved (the cc0 matmuls only need the first half of
                # h2), per-d-chunk epilogue + store on parallel queues
                pses = [ps_pool.tile([P, 1024], F32, tag="ps", name="psd%d" % dc)
                        for dc in range(2)]
                for dc in range(2 if DRAIN_S3_ILV else 0):
                    nc.tensor.matmul(pses[dc][:, :HW],
                                     w2sb[:, 0, dc * 128 : (dc + 1) * 128],
                                     h2[:, 0, :], start=True, stop=False)
                for dc in range(2):
                    if not DRAIN_S3_ILV:
                        nc.tensor.matmul(pses[dc][:, :HW],
                                         w2sb[:, 0, dc * 128 : (dc + 1) * 128],
                                         h2[:, 0, :], start=True, stop=False)
                    out = pses[dc][:, :HW]
                    nc.tensor.matmul(out,
                                     w2sb[:, 1, dc * 128 : (dc + 1) * 128],
                                     h2[:, 1, :], start=False, stop=True)
                    if b3sb is not None:
                        nc.scalar.activation(
                            yo[:, k, dc, :], out,
                            AF.Relu, bias=b3sb[:, dc : dc + 1])
                        nc.vector.tensor_tensor(
                            yo[:, k, dc, :], yo[:, k, dc, :], xbt[:, k, dc, :],
                            ALU.add)
                    else:
                        nc.vector.scalar_tensor_tensor(
                            yo[:, k, dc, :], out,
                            0.0, xbt[:, k, dc, :], ALU.max, ALU.add)
                    (nc.sync if dc == 0 else nc.scalar).dma_start(
                        yc[i, :, dc, :], yo[:, k, dc, :])

            def emit_s3_group(i, k, yo, h2, dc, pss, xbt, drain=False):
                if drain:
                    if dc == 0:
                        emit_s3_drain(i, k, yo, h2, xbt)
                    return
                ps = pss["s3"]
                out = ps[:, dc * 512 : dc * 512 + HW]
                dsl = slice(dc * 128, (dc + 1) * 128)
                nc.tensor.matmul(out, w2sb[:, 0, dsl], h2[:, 0, :],
                                 start=True, stop=False)
                nc.tensor.matmul(out, w2sb[:, 1, dsl], h2[:, 1, :],
                                 start=False, stop=True)
                if dc == 0:
                    return
                psv = ps.rearrange("p (c x) -> p c x", c=2)[:, :, :HW]
                if b3sb is not None:
                    for d2 in range(2):
                        nc.scalar.activation(
                            yo[:, k, d2, :], psv[:, d2, :], AF.Relu,
                            bias=b3sb[:, d2 : d2 + 1])
                    nc.vector.tensor_tensor(
                        yo[:, k, :, :], yo[:, k, :, :], xbt[:, k, :, :],
                        ALU.add)
                else:
                    nc.vector.scalar_tensor_tensor(
                        yo[:, k, :, :], psv, 0.0, xbt[:, k, :, :],
                        ALU.max, ALU.add)

            def emit_store(bi, yo):
                imgs = BATCHES[bi]
                nb = len(imgs)
                # SWDGE path keeps store DMAs off the sync queue so they never
                # head-of-line-block prefetch loads
                nc.gpsimd.dma_start(
                    yc[imgs[0] : imgs[0] + nb].rearrange("n p c q -> p n c q"),
                    yo[:, :nb])

            def body():
                s1f, res = {}, {}
                h1s, h2s, yos = {}, {}, {}

                def load_x8(bi, queue=None):
                    imgs = BATCHES[bi]
                    nb = len(imgs)
                    xf = xf_pool.tile([P, BMAX, 4, XW], F8, tag="xf", name="xf")
                    src = x8[imgs[0] : imgs[0] + nb].rearrange("n p s q -> p n s q")
                    (queue or nc.sync).dma_start(xf[:, :nb], src)
                    for k, i in enumerate(imgs):
                        s1f[i] = (xf, k)

                def load_xb(bi):
                    imgs = BATCHES[bi]
                    nb = len(imgs)
                    xbt = xb_pool.tile([P, BMAX, 2, HW], BF16, tag="xb", name="xb")
                    nc.sync.dma_start(
                        xbt[:, :nb],
                        xb[imgs[0] : imgs[0] + nb].rearrange("n p c q -> p n c q"))
                    for k, i in enumerate(imgs):
                        res[i] = (xbt, k)

                # startup loads all ride the sync queue in need-order (w1 went
                # out on scalar before anything else; more scalar-queue DMAs
                # would stall ACT epilogue dispatch behind their ~1.2us SEQ
                # configs).  dw slots in before batches 2/3: s2(0) needs it
                # around t=7us.
                loaded = 0
                for tok, qn in LOAD_PLAN:
                    q = {"sy": nc.sync, "sc": nc.scalar,
                         "gp": nc.gpsimd}[qn]
                    if tok == "dwh":
                        q.dma_start(dwsb[:, 0:6], dwb[:, 0:6])
                    elif tok == "dwl":
                        q.dma_start(dwsb[:, 6:10], dwb[:, 6:10])
                    elif tok == "dw":
                        q.dma_start(dwsb[:], dwb)
                    elif tok == "w2":
                        q.dma_start(w2sb[:], w2b)
                    else:
                        load_x8(tok, queue=q)
                        loaded = tok + 1
                if b1sb is not None:
                    nc.sync.dma_start(b1sb[:], b1)
                if b2sb is not None:
                    nc.sync.dma_start(b2sb[:], b2)
                if b3sb is not None:
                    nc.sync.dma_start(b3sb[:], b3.rearrange("co ci -> ci co"))
                xb_loaded = 0
                def alloc_ps(name):
                    return ps_pool.tile([P, 1024], F32, tag="ps", name=name)

                def setup_i3(i3):
                    bi3, k3 = batch_of[i3]
                    if k3 == 0:
                        yos[bi3] = yo_pool.tile(
                            [P, BMAX, 2, HW], BF16, tag="yo", name="yo")
                    return bi3, k3

                def finish_i3(i3, bi3, k3):
                    h2s.pop(i3)
                    s1f.pop(i3, None)
                    if k3 == len(BATCHES[bi3]) - 1 and i3 < NIMG - 2:
                        emit_store(bi3, yos.pop(bi3))

                for step in range(NIMG + 3):
                    if step % 2 == 0 and loaded < len(BATCHES):
                        load_x8(loaded)
                        loaded += 1
                        while xb_loaded < loaded - 2:
                            load_xb(xb_loaded)
                            xb_loaded += 1
                    if step >= NIMG - 2:
                        while xb_loaded < loaded:
                            load_xb(xb_loaded)
                            xb_loaded += 1
                    i1 = step if step < NIMG else None
                    i2 = step - 2 if 2 <= step < NIMG + 2 else None
                    i3 = step - 3 if step >= 3 else None
                    pss = {}
                    if i1 is not None:
                        pss["s1"] = alloc_ps("ps1")
                    if i2 is not None:
                        pss["s2a"] = ps2_pool.tile([P, 512], F32, tag="s2ps",
                                                   name="s2a")
                        pss["s2b"] = ps2_pool.tile([P, 512], F32, tag="s2ps",
                                                   name="s2b")
                    if i3 is not None and i3 < NIMG - 2:
                        pss["s3"] = alloc_ps("ps3")
                    if i1 is not None:
                        h1s[i1] = h1_pool.tile([P, 6, C], F8, tag="h1", name="h1")
                    bi3 = k3 = None
                    if i3 is not None:
                        bi3, k3 = setup_i3(i3)
                    if i2 is not None:
                        h2s[i2] = h2_pool.tile([P, 2, HW], BF16, tag="h2", name="h2")
                    order = STEP_ORDER if step >= EARLY_CUT else EARLY_ORDER
                    for stg, g in order:
                        if stg == 1 and i1 is not None:
                            xf, k1 = s1f[i1]
                            emit_s1_group(i1, xf, k1, g, h1s[i1], pss)
                        elif stg == 3 and i3 is not None:
                            emit_s3_group(i3, k3, yos[bi3], h2s[i3], g, pss,
                                          res[i3][0], drain=(i3 >= NIMG - 2))
                        elif stg == 2 and i2 is not None:
                            emit_s2_group(i2, h1s[i2], g, h2s[i2], pss)
                    if i2 is not None:
                        h1s.pop(i2)
                    if i3 is not None:
                        finish_i3(i3, bi3, k3)

            if reps == 1:
                body()
            else:
                with tc.For_i(0, reps, 1):
                    body()

    nc.compile()
    return nc


def _get_program(key):
    if key not in _prog_cache:
        _prog_cache[key] = build_program(*key)
    return _prog_cache[key]


def _marshal(x, conv1_w, conv1_b, bn1_mean, bn1_var, bn1_beta,
             dense_w, dense_b, conv2_w, conv2_b, bn2_mean, bn2_var, bn2_beta):
    bf16 = ml_dtypes.bfloat16
    f8 = ml_dtypes.float8_e4m3
    n = x.shape[0]
    rs1 = 1.0 / np.sqrt(bn1_var.astype(np.float64) + EPS)
    rs2 = 1.0 / np.sqrt(bn2_var.astype(np.float64) + EPS)
    w1f = conv1_w.astype(np.float64) * rs1[None, :]
    w2f = conv2_w.astype(np.float64) * rs2[None, :]
    b1f = (conv1_b - bn1_mean).astype(np.float64) * rs1 + bn1_beta
    b2f = dense_b.astype(np.float64)
    b3f = (conv2_b - bn2_mean).astype(np.float64) * rs2 + bn2_beta
    has_b1 = bool(np.any(b1f != 0.0))
    has_b2 = bool(np.any(b2f != 0.0))
    has_b3 = bool(np.any(b3f != 0.0))

    def hilo(a):
        h = a.astype(f8)
        l = (a - h.astype(a.dtype)).astype(f8)
        return h, l

    # w1 blob [P, 4, C]: slots [w1h_c0, w1h_c1, w1l_c0, w1l_c1]
    w1h, w1l = hilo(w1f)
    w1hr = w1h.reshape(2, P, C)
    w1lr = w1l.reshape(2, P, C)
    w1blob = np.ascontiguousarray(
        np.stack([w1hr[0], w1hr[1], w1lr[0], w1lr[1]], axis=1))

    # dw blob [P, 10, XW]: [dwh0 dwh1 dwh2 dwh0 dwh1 dwh2 dwl0 dwl1 dwl2 0],
    # q zero-padded to XW for an even lhsT/rhs slot stride
    dwp = np.zeros((3 * P, XW), np.float64)
    dwp[:HW, :HW] = dense_w
    dwh, dwl = hilo(dwp)
    dwhr = dwh.reshape(3, P, XW)
    dwlr = dwl.reshape(3, P, XW)
    zslot = np.zeros((P, XW), f8)
    s5 = dwhr[2] if LO_TILES == 3 else zslot   # lh2 slot unwritten when
    dwblob = np.ascontiguousarray(np.stack(      # LO_TILES==2: zero its dw
        [dwhr[0], dwhr[1], dwhr[2], dwhr[0], dwhr[1], s5,
         dwlr[0], dwlr[1], dwlr[2], zslot], axis=1))

    # w2 blob [P, 2, C] bf16
    w2r = w2f.reshape(2, P, C)
    w2blob = np.ascontiguousarray(np.stack([w2r[0], w2r[1]], axis=1)).astype(bf16)

    # x: [n, C, HW] fp32 (C-major), hi/lo fp8 (q padded to XW) + bf16 residual
    xC = np.ascontiguousarray(
        x.reshape(n, HW, C).transpose(0, 2, 1)).astype(np.float32)
    xh = np.zeros((n, C, XW), f8)
    xl = np.zeros((n, C, XW), f8)
    xh[:, :, :HW] = xC.astype(f8)
    xl[:, :, :HW] = (xC - xh[:, :, :HW].astype(np.float32)).astype(f8)
    x8b = np.ascontiguousarray(np.stack(
        [xh[:, :P], xh[:, P:], xl[:, :P], xl[:, P:]], axis=2)
    ).reshape(N_CORES, NIMG, P, 4, XW)
    xbb = np.ascontiguousarray(
        xC.astype(bf16).reshape(n, 2, P, HW).transpose(0, 2, 1, 3)
    ).reshape(N_CORES, NIMG, P, 2, HW)

    in_maps = []
    for cidx in range(N_CORES):
        m = {"x8": x8b[cidx], "xb": xbb[cidx],
             "w1b": w1blob, "dwb": dwblob, "w2b": w2blob}
        if has_b1:
            m["b1"] = np.ascontiguousarray(np.broadcast_to(
                np.tile(b1f, 3).astype(np.float32), (P, 3 * C)))
        if has_b2:
            m["b2"] = np.ascontiguousarray(np.broadcast_to(
                b2f.astype(np.float32), (P, 2, HW)))
        if has_b3:
            m["b3"] = np.ascontiguousarray(
                b3f.astype(np.float32).reshape(2, P))
        in_maps.append(m)
    return (has_b1, has_b2, has_b3), in_maps


def _unmarshal(results, n, h, w):
    y = np.stack([results[c]["yc"] for c in range(N_CORES)])
    # [cores, NIMG, P, 2, HW] bf16 -> [n, 2, P, HW] -> [n, C, HW]
    y = y.reshape(n, P, 2, HW).transpose(0, 2, 1, 3).reshape(n, C, HW)
    y = y.transpose(0, 2, 1).astype(np.float32)
    return np.ascontiguousarray(y.reshape(n, h, w, C))


def kernel(x, conv1_w, conv1_b, bn1_mean, bn1_var, bn1_beta,
           dense_w, dense_b, conv2_w, conv2_b, bn2_mean, bn2_var, bn2_beta):
    n, h, w, _ = x.shape
    flags, in_maps = _marshal(
        x, conv1_w, conv1_b, bn1_mean, bn1_var, bn1_beta,
        dense_w, dense_b, conv2_w, conv2_b, bn2_mean, bn2_var, bn2_beta)
    nc = _get_program((*flags, 1))
    res = run_bass_kernel_spmd(nc, in_maps, list(range(N_CORES)))
    return _unmarshal(res.results, n, h, w)
